# revision 16
# baseline (speedup 1.0000x reference)
"""Trainium2 Bass kernel for nn_BaseTimeAttention (dense transformer block:
QKV projection + RoPE + softmax attention + output projection).

Problem (hardcoded):
  x:  [B=2, S=2048, H=2048] fp32,  Wq/Wk/Wv/Wo: [2048, 2048] fp32
  out = softmax((rope(xWq^T) rope(xWk^T)^T)/sqrt(128)) (xWv^T) Wo^T

Sharding (8 cores): tensor-parallel over heads x data-parallel over batch.
Core c handles batch b=c//4 and head group g=c%4 (4 of 16 heads = 512 of 2048
channels). Each core produces a full [2048, 2048] fp16 partial of the output
projection restricted to its 512 input channels; the host sums 4 partials per
batch in fp32 (o_proj row-parallel reduction on host).

All compute runs in fp16 operands with fp32 PSUM accumulation. Q/K/V live
entirely in SBUF between the projection and attention phases. Inputs stream
on four dedicated DGE rings (x on scalar, wq on gpsimd, wk/wo on sync,
cos/sin/wv on vector) so the startup transient is bandwidth-, not
ordering-limited.

RoPE rotate-half needs a 64-partition swap: for s-block 0 (when the PE is
transfer-starved anyway) it runs as a permutation-matmul on the PE; later
blocks use SBUF->SBUF DMAs on the sync ring. The sign lives in the
host-built sin table.

Attention per (h, n) block (n a 512-wide query block):
  scoresT[s_k, s_q] = K-tile^T @ QT      (16 MMs; pairs share a 2-bank PSUM)
  eT = exp(scoresT / sqrt(128))          (ScalarE, one ACTIVATE per pair)
  num[d, s_q] += V-tile^T @ eT           (PE, accumulate over s_k)
  esum += eT                             (Pool: tiles 0-4, DVE: 5-15 + merge)
  den = ones^T @ esum                    (PE, 1 MM - replaces 16 ones-MMs)
  yt = num * recip(den)                  (VectorE, approx reciprocal)
Block epilogues (den/recip/mul) are software-pipelined into the next block's
score phase, and o_proj work is spread evenly over all blocks (4 PSUM groups
per block, woven in at pipeline steps 3 and 6) so the PE always has runnable
work while the ScalarE exp stream - the co-bottleneck - catches up.
"""

import numpy as np

import concourse.mybir as mybir
import concourse.tile as tile
from concourse import bacc
from concourse.bass_utils import run_bass_kernel_spmd

F32 = mybir.dt.float32
F16 = mybir.dt.float16
AF = mybir.ActivationFunctionType

B = 2
S = 2048
HIDDEN = 2048
HEADS = 16
DH = 128
THETA = 10000.0
N_CORES = 8
GROUPS = 4
HPC = HEADS // GROUPS  # heads per core
JPC = HPC * DH  # projection cols per core
SCALE = 1.0 / np.sqrt(DH)

SB = 512
NSB = S // SB
KT = HIDDEN // 128  # 16 contraction tiles
NKT = S // 128  # 16 s_k tiles
NP = NKT // 2  # 8 score pairs per block
PIPE = 2

CHS = (1, 1, 2, 4, 8)  # phase-1 k-chunk sizes (finest first for fast start)
CHO = (0, 1, 2, 4, 8)


def build():
    nc = bacc.Bacc("TRN2", target_bir_lowering=False, debug=False)

    # partition-major inputs (see _make_in_maps)
    x_d = nc.dram_tensor("xPM", [NSB, 128, KT, SB], F16, kind="ExternalInput")
    wq_d = nc.dram_tensor("wqPM", [128, KT, JPC], F16, kind="ExternalInput")
    wk_d = nc.dram_tensor("wkPM", [128, KT, JPC], F16, kind="ExternalInput")
    wv_d = nc.dram_tensor("wvPM", [128, KT, JPC], F16, kind="ExternalInput")
    wo_d = nc.dram_tensor("woPM", [128, HPC, HIDDEN], F16, kind="ExternalInput")
    cos_d = nc.dram_tensor("cos", [DH, S], F16, kind="ExternalInput")
    sin_d = nc.dram_tensor("sinS", [DH, S], F16, kind="ExternalInput")
    ones_d = nc.dram_tensor("ones", [128, 128], F16, kind="ExternalInput")
    pswap_d = nc.dram_tensor("pswap", [128, 128], F16, kind="ExternalInput")
    out_d = nc.dram_tensor("out", [S, HIDDEN], F16, kind="ExternalOutput")

    out = out_d.ap()

    with tile.TileContext(nc) as tc:
        with tc.tile_pool(name="persist", bufs=1) as persist:
            q_sb = persist.tile([128, HPC, S], F16, tag="q_sb")
            k_sb = persist.tile([128, HPC, S], F16, tag="k_sb")
            v_sb = persist.tile([128, HPC, NKT, DH], F16, tag="v_sb")
            wo = persist.tile([128, HPC, HIDDEN], F16, tag="wo")
            ones_sb = persist.tile([128, 128], F16, tag="ones")
            pswap = persist.tile([128, 128], F16, tag="pswap")
            nc.sync.dma_start(out=ones_sb[:], in_=ones_d.ap())
            nc.sync.dma_start(out=pswap[:], in_=pswap_d.ap())

            # ---------------- Phase 1: projections + RoPE ------------------
            def wslice(chunks, k, cols):
                if len(chunks) == 1:  # full-KT tile
                    return chunks[0][:, k, cols]
                for c in range(len(CHS)):
                    if k < CHO[c] + CHS[c]:
                        return chunks[c][:, k - CHO[c], cols]
                raise AssertionError

            with (
                tc.tile_pool(name="p1w", bufs=1) as p1w,
                tc.tile_pool(name="p1x", bufs=1) as p1x,
                tc.tile_pool(name="p1cs", bufs=1) as p1cs,
                tc.tile_pool(name="p1s", bufs=4) as p1s,
                tc.tile_pool(name="p1ps", bufs=2, space="PSUM") as p1ps,
                tc.tile_pool(name="p1rot", bufs=2, space="PSUM") as p1rot,
            ):
                # three DGE rings, each input class placed so the startup is
                # bandwidth-limited, not ordering-limited:
                #   scalar: x      gpsimd: wq, sin, wv
                #   sync: consts, cos, wk, wo (+ swaps, out)
                wchunks = {}
                for name, w_d, eng in (
                    ("q", wq_d, nc.gpsimd),
                    ("k", wk_d, nc.sync),
                ):
                    for c in range(len(CHS)):
                        w = p1w.tile([128, CHS[c], JPC], F16, tag=f"w{name}{c}")
                        eng.dma_start(
                            out=w[:],
                            in_=w_d.ap()[:, CHO[c] : CHO[c] + CHS[c], :],
                        )
                        wchunks.setdefault(name, []).append(w)

                def load_xs(s):
                    if s == 0:
                        xsc = []
                        for c in range(len(CHS)):
                            xt = p1x.tile(
                                [128, CHS[c], SB], F16, tag=f"xs{c}"
                            )
                            nc.scalar.dma_start(
                                out=xt[:],
                                in_=x_d.ap()[s, :, CHO[c] : CHO[c] + CHS[c], :],
                            )
                            xsc.append(xt)
                        return xsc
                    xt = p1x.tile([128, KT, SB], F16, tag="xsf", bufs=2)
                    nc.scalar.dma_start(out=xt[:], in_=x_d.ap()[s])
                    return [xt]

                xs_next = load_xs(0)
                cos_sb = p1cs.tile([128, S], F16, tag="cos")
                sin_sb = p1cs.tile([128, S], F16, tag="sin")
                nc.sync.dma_start(out=cos_sb[:], in_=cos_d.ap())
                nc.sync.dma_start(out=sin_sb[:], in_=sin_d.ap())
                for c in range(len(CHS)):
                    w = p1w.tile([128, CHS[c], JPC], F16, tag=f"wv{c}")
                    nc.gpsimd.dma_start(
                        out=w[:], in_=wv_d.ap()[:, CHO[c] : CHO[c] + CHS[c], :]
                    )
                    wchunks.setdefault("v", []).append(w)
                # s0 uses the PE to produce the rotate-half partner (the PE
                # is transfer-starved early on, so rot-MMs are free); later
                # s-blocks use sync-ring SBUF->SBUF swap DMAs.
                pending_rot = []

                def flush_rot():
                    while pending_rot:
                        qt, dst, j, sblk = pending_rot.pop(0)
                        rot = p1rot.tile([128, SB], F32, tag="rot")
                        nc.tensor.matmul(
                            rot[:], pswap[:], qt[:], start=True, stop=True
                        )
                        t1 = p1s.tile([128, SB], F16, tag="t1")
                        nc.vector.tensor_mul(t1[:], rot[:], sin_sb[:, sblk])
                        nc.vector.tensor_mul(qt[:], qt[:], cos_sb[:, sblk])
                        nc.vector.tensor_add(dst[:, j, sblk], qt[:], t1[:])

                for s in range(NSB):
                    sblk = slice(s * SB, (s + 1) * SB)
                    xsc = xs_next
                    for name, dst in (("q", q_sb), ("k", k_sb), ("v", None)):
                        if name == "k" and s + 1 < NSB:
                            xs_next = load_xs(s + 1)
                        if name == "v" and s == 0:
                            # o_proj weights ride the sync ring behind wk
                            for kj in range(HPC):
                                nc.sync.dma_start(
                                    out=wo[:, kj, :], in_=wo_d.ap()[:, kj, :]
                                )
                        for j in range(HPC):
                            jblk = slice(j * 128, (j + 1) * 128)
                            if dst is not None:
                                ps = p1ps.tile([128, SB], F32, tag="ps")
                            else:
                                ps = p1ps.tile(
                                    [128, HPC, DH], F32, tag="ps", name="psv"
                                )
                            for k in range(KT):
                                if dst is not None:  # Q/K: [j, s] transposed
                                    lhsT = wslice(wchunks[name], k, jblk)
                                    rhs = wslice(xsc, k, slice(0, SB))
                                else:  # V: natural [s, j]
                                    lhsT = wslice(xsc, k, jblk)
                                    rhs = wslice(wchunks[name], k, slice(0, JPC))
                                nc.tensor.matmul(
                                    ps[:],
                                    lhsT,
                                    rhs,
                                    start=(k == 0),
                                    stop=(k == KT - 1),
                                )
                            if s == 0:
                                flush_rot()
                            if dst is not None:
                                qt = p1s.tile([128, SB], F16, tag="qt")
                                nc.scalar.copy(qt[:], ps[:])
                                if s == 0:
                                    pending_rot.append((qt, dst, j, sblk))
                                else:
                                    tmp = p1s.tile([128, SB], F16, tag="tmp")
                                    nc.sync.dma_start(
                                        out=tmp[0:64, :], in_=qt[64:128, :]
                                    )
                                    nc.sync.dma_start(
                                        out=tmp[64:128, :], in_=qt[0:64, :]
                                    )
                                    nc.vector.tensor_mul(
                                        qt[:], qt[:], cos_sb[:, sblk]
                                    )
                                    nc.vector.tensor_mul(
                                        tmp[:], tmp[:], sin_sb[:, sblk]
                                    )
                                    nc.vector.tensor_add(
                                        dst[:, j, sblk], qt[:], tmp[:]
                                    )
                            else:
                                # V tile covers s rows [s*SB+j*128 ...), all
                                # HPC heads; scatter heads into v_sb
                                nc.scalar.copy(
                                    v_sb[:, :, s * HPC + j, :], ps[:]
                                )
                    if s == 0:
                        flush_rot()

            # ---------------- Phase 2: attention + o_proj ------------------
            with (
                tc.tile_pool(name="p2y", bufs=2) as p2y,
                tc.tile_pool(name="p2e", bufs=4) as p2e,
                tc.tile_pool(name="p2acc", bufs=2) as p2acc,
                tc.tile_pool(name="p2r", bufs=2) as p2r,
                tc.tile_pool(name="p2oc", bufs=4) as p2oc,
                tc.tile_pool(name="p2sc", bufs=2, space="PSUM") as p2sc,
                tc.tile_pool(name="p2num", bufs=2, space="PSUM") as p2num,
                tc.tile_pool(name="p2aux", bufs=2, space="PSUM") as p2aux,
            ):
                yts = {}

                def emit_epi(epi):
                    n, h, num, acc = epi
                    den = p2aux.tile([128, SB], F32, tag="aux", name="den")
                    nc.tensor.matmul(
                        den[:], ones_sb[:], acc[:], start=True, stop=True
                    )
                    r = p2r.tile([128, SB], F32, tag="r")
                    nc.vector.reciprocal_approx_fast(out=r[:], in_=den[:])
                    nc.vector.tensor_mul(yts[n][:, h, :], num[:], r[:])

                oproj_count = [0]

                def emit_oproj_group(n, m, oc_i, tail=False):
                    yt = yts[n]
                    mrow = n * SB + m * 128
                    ocb = slice(oc_i * SB, (oc_i + 1) * SB)
                    pso = p2aux.tile([128, SB], F32, tag="aux", name="pso")
                    for kj in range(HPC):
                        nc.tensor.matmul(
                            pso[:],
                            yt[:, kj, m * 128 : (m + 1) * 128],
                            wo[:, kj, ocb],
                            start=(kj == 0),
                            stop=(kj == HPC - 1),
                        )
                    occ = p2oc.tile([128, SB], F16, tag="oc")
                    i = oproj_count[0]
                    oproj_count[0] += 1
                    nc.vector.tensor_scalar_mul(occ[:], pso[:], 1.0)
                    eng = nc.scalar if (tail and i % 2) else nc.sync
                    eng.dma_start(out=out[mrow : mrow + 128, ocb], in_=occ[:])

                pending_epi = None
                oproj_fifo = []  # (n, m, oc_i) groups ready to weave in
                for n in range(NSB):
                    nblk = slice(n * SB, (n + 1) * SB)
                    for h in range(HPC):
                        if h == 0:
                            yts[n] = p2y.tile(
                                [128, HPC, SB], F16, tag="yt", name="yt"
                            )
                        acc_p = p2acc.tile([128, SB], F16, tag="accp")
                        acc_d = p2acc.tile([128, SB], F16, tag="accd")
                        num = p2num.tile([128, SB], F32, tag="num")
                        es = [None] * NP
                        for p in range(NP + PIPE):
                            if p < NP:
                                sc2 = p2sc.tile([128, 2, SB], F32, tag="sc")
                                e2 = p2e.tile([128, 2, SB], F16, tag="e")
                                for half in range(2):
                                    i = 2 * p + half
                                    nc.tensor.matmul(
                                        sc2[:, half, :],
                                        k_sb[:, h, i * 128 : (i + 1) * 128],
                                        q_sb[:, h, nblk],
                                        start=True,
                                        stop=True,
                                    )
                                nc.scalar.activation(
                                    e2[:], sc2[:], AF.Exp, scale=float(SCALE)
                                )
                                es[p] = e2
                                # esum: Pool sums e0-e5, DVE e6-e15; one
                                # merge at block end (late tiles on the
                                # faster engine so den isn't gated on Pool)
                                if p == 0:
                                    nc.gpsimd.tensor_add(
                                        acc_p[:], e2[:, 0, :], e2[:, 1, :]
                                    )
                                elif p <= 2:
                                    for half in range(2):
                                        nc.gpsimd.tensor_add(
                                            acc_p[:], acc_p[:], e2[:, half, :]
                                        )
                                elif p == 3:
                                    nc.vector.tensor_add(
                                        acc_d[:], e2[:, 0, :], e2[:, 1, :]
                                    )
                                else:
                                    for half in range(2):
                                        nc.vector.tensor_add(
                                            acc_d[:], acc_d[:], e2[:, half, :]
                                        )
                            if p == 1 and pending_epi is not None:
                                emit_epi(pending_epi)
                                pending_epi = None
                            if p in (3, 6):
                                for _ in range(2):
                                    if oproj_fifo:
                                        emit_oproj_group(*oproj_fifo.pop(0))
                            if p >= PIPE:
                                pp = p - PIPE
                                for half in range(2):
                                    i = 2 * pp + half
                                    nc.tensor.matmul(
                                        num[:],
                                        v_sb[:, h, i, :],
                                        es[pp][:, half, :],
                                        start=(i == 0),
                                        stop=(i == NKT - 1),
                                    )
                        nc.vector.tensor_add(acc_d[:], acc_d[:], acc_p[:])
                        pending_epi = (n, h, num, acc_d)
                    # after epi(n, h3) is emitted (at the next block's p1),
                    # n's o_proj groups become weavable; enqueue now - the
                    # first pop happens at that block's p3, after the epi
                    oproj_fifo.extend(
                        (n, m, oc_i)
                        for m in range(SB // 128)
                        for oc_i in range(HIDDEN // SB)
                    )

                emit_epi(pending_epi)
                while oproj_fifo:
                    emit_oproj_group(*oproj_fifo.pop(0), tail=True)

    nc.compile()
    return nc


_NC = None


def _get_nc():
    global _NC
    if _NC is None:
        _NC = build()
    return _NC


def _rope_tables():
    inv_freq = 1.0 / (THETA ** (np.arange(0, DH, 2, dtype=np.float32) / DH))
    freqs = np.arange(S, dtype=np.float32)[:, None] * inv_freq[None, :]  # [S, 64]
    cos_h = np.cos(freqs).T.astype(np.float32)  # [64, S]
    sin_h = np.sin(freqs).T.astype(np.float32)
    cos = np.concatenate([cos_h, cos_h], axis=0)  # [128, S]
    sin_s = np.concatenate([-sin_h, sin_h], axis=0)  # [128, S]
    return np.ascontiguousarray(cos), np.ascontiguousarray(sin_s)


def _pm_weight(wT):  # [2048, 512] (k, j) -> [128, 16, 512] partition-major
    return np.ascontiguousarray(
        wT.reshape(KT, 128, JPC).transpose(1, 0, 2)
    ).astype(np.float16)


def _make_in_maps(inputs):
    x = np.asarray(inputs["x"], dtype=np.float32)
    Wq = np.asarray(inputs["Wq"], dtype=np.float32)
    Wk = np.asarray(inputs["Wk"], dtype=np.float32)
    Wv = np.asarray(inputs["Wv"], dtype=np.float32)
    Wo = np.asarray(inputs["Wo"], dtype=np.float32)

    cos, sin_s = _rope_tables()
    cos = cos.astype(np.float16)
    sin_s = sin_s.astype(np.float16)
    ones = np.ones((128, 128), dtype=np.float16)
    pswap = np.zeros((128, 128), dtype=np.float16)
    pswap[(np.arange(128) + 64) % 128, np.arange(128)] = 1.0

    in_maps = []
    for c in range(N_CORES):
        b = c // GROUPS
        g = c % GROUPS
        rows = slice(g * JPC, (g + 1) * JPC)
        xT = x[b].T  # [hidden(k), s]
        # [k, s] -> [s_blk, p, kt, s_in_blk]
        xpm = np.ascontiguousarray(
            xT.reshape(KT, 128, NSB, SB).transpose(2, 1, 0, 3)
        ).astype(np.float16)
        # Wo[:, rows].T -> [512(j), 2048] -> [p, kj, 2048]
        woT = Wo[:, rows].T
        wopm = np.ascontiguousarray(
            woT.reshape(HPC, 128, HIDDEN).transpose(1, 0, 2)
        ).astype(np.float16)
        in_maps.append(
            {
                "xPM": xpm,
                "wqPM": _pm_weight(Wq[rows].T),
                "wkPM": _pm_weight(Wk[rows].T),
                "wvPM": _pm_weight(Wv[rows].T),
                "woPM": wopm,
                "cos": cos,
                "sinS": sin_s,
                "ones": ones,
                "pswap": pswap,
            }
        )
    return in_maps


def kernel(x, Wq, Wk, Wv, Wo):
    nc = _get_nc()
    in_maps = _make_in_maps({"x": x, "Wq": Wq, "Wk": Wk, "Wv": Wv, "Wo": Wo})
    res = run_bass_kernel_spmd(nc, in_maps, list(range(N_CORES)))

    out = np.zeros((B, S, HIDDEN), dtype=np.float32)
    for c in range(N_CORES):
        out[c // GROUPS] += res.results[c]["out"].astype(np.float32)
    return out


# revision 17
# speedup vs baseline: 1.0371x; 1.0371x over previous
"""Trainium2 Bass kernel for nn_BaseTimeAttention (dense transformer block:
QKV projection + RoPE + softmax attention + output projection).

Problem (hardcoded):
  x:  [B=2, S=2048, H=2048] fp32,  Wq/Wk/Wv/Wo: [2048, 2048] fp32
  out = softmax((rope(xWq^T) rope(xWk^T)^T)/sqrt(128)) (xWv^T) Wo^T

Sharding (8 cores): tensor-parallel over heads x data-parallel over batch.
Core c handles batch b=c//4 and head group g=c%4 (4 of 16 heads = 512 of 2048
channels). Each core produces a full [2048, 2048] fp16 partial of the output
projection restricted to its 512 input channels; the host sums 4 partials per
batch in fp32 (o_proj row-parallel reduction on host).

All compute runs in fp16 operands with fp32 PSUM accumulation. Q/K/V live
entirely in SBUF between the projection and attention phases. Inputs stream
on four dedicated DGE rings (x on scalar, wq on gpsimd, wk/wo on sync,
cos/sin/wv on vector) so the startup transient is bandwidth-, not
ordering-limited.

RoPE rotate-half needs a 64-partition swap: for s-block 0 (when the PE is
transfer-starved anyway) it runs as a permutation-matmul on the PE; later
blocks use SBUF->SBUF DMAs on the sync ring. The sign lives in the
host-built sin table.

Attention per (h, n) block (n a 512-wide query block):
  scoresT[s_k, s_q] = K-tile^T @ QT      (16 MMs; pairs share a 2-bank PSUM)
  eT = exp(scoresT / sqrt(128))          (ScalarE, one ACTIVATE per pair)
  num[d, s_q] += V-tile^T @ eT           (PE, accumulate over s_k)
  esum += eT                             (Pool: tiles 0-4, DVE: 5-15 + merge)
  den = ones^T @ esum                    (PE, 1 MM - replaces 16 ones-MMs)
  yt = num * recip(den)                  (VectorE, approx reciprocal)
Block epilogues (den/recip/mul) are software-pipelined into the next block's
score phase, and o_proj work is spread evenly over all blocks (4 PSUM groups
per block, woven in at pipeline steps 3 and 6) so the PE always has runnable
work while the ScalarE exp stream - the co-bottleneck - catches up.
"""

import numpy as np

import concourse.mybir as mybir
import concourse.tile as tile
from concourse import bacc
from concourse.bass_utils import run_bass_kernel_spmd

F32 = mybir.dt.float32
F16 = mybir.dt.float16
AF = mybir.ActivationFunctionType

B = 2
S = 2048
HIDDEN = 2048
HEADS = 16
DH = 128
THETA = 10000.0
N_CORES = 8
GROUPS = 4
HPC = HEADS // GROUPS  # heads per core
JPC = HPC * DH  # projection cols per core
SCALE = 1.0 / np.sqrt(DH)

SB = 512
NSB = S // SB
KT = HIDDEN // 128  # 16 contraction tiles
NKT = S // 128  # 16 s_k tiles
NP = NKT // 2  # 8 score pairs per block
PIPE = 2

CHS = (1, 1, 2, 4, 8)  # phase-1 k-chunk sizes (finest first for fast start)
CHO = (0, 1, 2, 4, 8)


def build():
    nc = bacc.Bacc("TRN2", target_bir_lowering=False, debug=False)

    # partition-major inputs (see _make_in_maps)
    x_d = nc.dram_tensor("xPM", [NSB, 128, KT, SB], F16, kind="ExternalInput")
    wq_d = nc.dram_tensor("wqPM", [128, KT, JPC], F16, kind="ExternalInput")
    wk_d = nc.dram_tensor("wkPM", [128, KT, JPC], F16, kind="ExternalInput")
    wv_d = nc.dram_tensor("wvPM", [128, KT, JPC], F16, kind="ExternalInput")
    wo_d = nc.dram_tensor("woPM", [128, HPC, HIDDEN], F16, kind="ExternalInput")
    cos_d = nc.dram_tensor("cos", [DH, S], F16, kind="ExternalInput")
    sin_d = nc.dram_tensor("sinS", [DH, S], F16, kind="ExternalInput")
    ones_d = nc.dram_tensor("ones", [128, 128], F16, kind="ExternalInput")
    pswap_d = nc.dram_tensor("pswap", [128, 128], F16, kind="ExternalInput")
    out_d = nc.dram_tensor("out", [S, HIDDEN], F16, kind="ExternalOutput")

    out = out_d.ap()

    with tile.TileContext(nc) as tc:
        with tc.tile_pool(name="persist", bufs=1) as persist:
            q_sb = persist.tile([128, HPC, S], F16, tag="q_sb")
            k_sb = persist.tile([128, HPC, S], F16, tag="k_sb")
            v_sb = persist.tile([128, HPC, NKT, DH], F16, tag="v_sb")
            wo = persist.tile([128, HPC, HIDDEN], F16, tag="wo")
            ones_sb = persist.tile([128, 128], F16, tag="ones")
            pswap = persist.tile([128, 128], F16, tag="pswap")
            nc.sync.dma_start(out=ones_sb[:], in_=ones_d.ap())
            nc.sync.dma_start(out=pswap[:], in_=pswap_d.ap())

            # ---------------- Phase 1: projections + RoPE ------------------
            def wslice(chunks, k, cols):
                for off, sz, t in chunks:
                    if off <= k < off + sz:
                        return t[:, k - off, cols]
                raise AssertionError

            with (
                tc.tile_pool(name="p1w", bufs=1) as p1w,
                tc.tile_pool(name="p1x", bufs=1) as p1x,
                tc.tile_pool(name="p1cs", bufs=1) as p1cs,
                tc.tile_pool(name="p1s", bufs=4) as p1s,
                tc.tile_pool(name="p1ps", bufs=2, space="PSUM") as p1ps,
                tc.tile_pool(name="p1rot", bufs=2, space="PSUM") as p1rot,
            ):
                # Wave-scheduled loads: x0 and wq (the first-wave inputs)
                # are split across all three DGE rings so the startup ramp is
                # aggregate-bandwidth-bound; wk/wv/wo follow in deadline
                # order. The Tile scheduler interleaves matmul groups
                # chunk-wise, so per-chunk arrival is what matters.
                #   scalar: x0/3rd, cos, wk-h2a, x1, x2, x3
                #   sync:   consts, x0/3rd, wq/3rd, sin, wv-h1 (+swaps, out)
                #   gpsimd: wq/3rd, wk-h1, wv-h2, wo
                cos_sb = p1cs.tile([128, S], F16, tag="cos")
                sin_sb = p1cs.tile([128, S], F16, tag="sin")

                def load_split(src_ap, tag, spec, pool, width):
                    lst = []
                    for idx, (eng, off, sz) in enumerate(spec):
                        t = pool.tile(
                            [128, sz, width], F16, tag=f"{tag}{idx}", name=tag
                        )
                        eng.dma_start(out=t[:], in_=src_ap[:, off : off + sz, :])
                        lst.append((off, sz, t))
                    return lst

                wchunks = {}
                xw1 = [
                    (nc.scalar, 0, 1),
                    (nc.scalar, 1, 1),
                    (nc.scalar, 2, 2),
                    (nc.scalar, 4, 2),
                    (nc.sync, 6, 2),
                    (nc.sync, 8, 2),
                    (nc.gpsimd, 10, 3),
                    (nc.gpsimd, 13, 3),
                ]
                wq1 = [
                    (nc.gpsimd, 0, 1),
                    (nc.gpsimd, 1, 1),
                    (nc.gpsimd, 2, 2),
                    (nc.gpsimd, 4, 2),
                    (nc.scalar, 6, 2),
                    (nc.scalar, 8, 2),
                    (nc.sync, 10, 3),
                    (nc.sync, 13, 3),
                ]
                xs0 = load_split(x_d.ap()[0], "xs0", xw1, p1x, SB)
                wchunks["q"] = load_split(wq_d.ap(), "wq", wq1, p1w, JPC)
                nc.scalar.dma_start(out=cos_sb[:], in_=cos_d.ap())
                nc.sync.dma_start(out=sin_sb[:], in_=sin_d.ap())
                wchunks["k"] = load_split(
                    wk_d.ap(),
                    "wk",
                    [
                        (nc.gpsimd, 0, 2),
                        (nc.gpsimd, 2, 2),
                        (nc.gpsimd, 4, 4),
                        (nc.scalar, 8, 4),
                        (nc.sync, 12, 4),
                    ],
                    p1w,
                    JPC,
                )
                wchunks["v"] = load_split(
                    wv_d.ap(),
                    "wv",
                    [
                        (nc.sync, 0, 2),
                        (nc.sync, 2, 2),
                        (nc.sync, 4, 4),
                        (nc.gpsimd, 8, 4),
                        (nc.gpsimd, 12, 4),
                    ],
                    p1w,
                    JPC,
                )
                for kj in range(HPC):
                    nc.gpsimd.dma_start(out=wo[:, kj, :], in_=wo_d.ap()[:, kj, :])

                def load_xs(s):
                    xt = p1x.tile([128, KT, SB], F16, tag="xsf", bufs=2)
                    nc.scalar.dma_start(out=xt[:], in_=x_d.ap()[s])
                    return [(0, KT, xt)]

                xs_next = xs0

                # s0 uses the PE to produce the rotate-half partner (the PE
                # is transfer-starved early on, so rot-MMs are free); later
                # s-blocks use sync-ring SBUF->SBUF swap DMAs.
                pending_rot = []

                def flush_rot():
                    while pending_rot:
                        qt, dst, j, sblk = pending_rot.pop(0)
                        rot = p1rot.tile([128, SB], F32, tag="rot")
                        nc.tensor.matmul(
                            rot[:], pswap[:], qt[:], start=True, stop=True
                        )
                        t1 = p1s.tile([128, SB], F16, tag="t1")
                        nc.vector.tensor_mul(t1[:], rot[:], sin_sb[:, sblk])
                        nc.vector.tensor_mul(qt[:], qt[:], cos_sb[:, sblk])
                        nc.vector.tensor_add(dst[:, j, sblk], qt[:], t1[:])

                for s in range(NSB):
                    sblk = slice(s * SB, (s + 1) * SB)
                    xsc = xs_next
                    for name, dst in (("q", q_sb), ("k", k_sb), ("v", None)):
                        if name == "k" and s + 1 < NSB:
                            xs_next = load_xs(s + 1)
                        for j in range(HPC):
                            jblk = slice(j * 128, (j + 1) * 128)
                            if dst is not None:
                                ps = p1ps.tile([128, SB], F32, tag="ps")
                            else:
                                ps = p1ps.tile(
                                    [128, HPC, DH], F32, tag="ps", name="psv"
                                )
                            for k in range(KT):
                                if dst is not None:  # Q/K: [j, s] transposed
                                    lhsT = wslice(wchunks[name], k, jblk)
                                    rhs = wslice(xsc, k, slice(0, SB))
                                else:  # V: natural [s, j]
                                    lhsT = wslice(xsc, k, jblk)
                                    rhs = wslice(wchunks[name], k, slice(0, JPC))
                                nc.tensor.matmul(
                                    ps[:],
                                    lhsT,
                                    rhs,
                                    start=(k == 0),
                                    stop=(k == KT - 1),
                                )
                            if s == 0:
                                flush_rot()
                            if dst is not None:
                                qt = p1s.tile(
                                    [128, SB], F16, tag="qt", bufs=6
                                )
                                nc.scalar.copy(qt[:], ps[:])
                                if s == 0:
                                    pending_rot.append((qt, dst, j, sblk))
                                else:
                                    tmp = p1s.tile([128, SB], F16, tag="tmp")
                                    nc.sync.dma_start(
                                        out=tmp[0:64, :], in_=qt[64:128, :]
                                    )
                                    nc.sync.dma_start(
                                        out=tmp[64:128, :], in_=qt[0:64, :]
                                    )
                                    nc.vector.tensor_mul(
                                        qt[:], qt[:], cos_sb[:, sblk]
                                    )
                                    nc.vector.tensor_mul(
                                        tmp[:], tmp[:], sin_sb[:, sblk]
                                    )
                                    nc.vector.tensor_add(
                                        dst[:, j, sblk], qt[:], tmp[:]
                                    )
                            else:
                                # V tile covers s rows [s*SB+j*128 ...), all
                                # HPC heads; scatter heads into v_sb
                                nc.scalar.copy(
                                    v_sb[:, :, s * HPC + j, :], ps[:]
                                )
                    if s == 0:
                        flush_rot()

            # ---------------- Phase 2: attention + o_proj ------------------
            with (
                tc.tile_pool(name="p2y", bufs=2) as p2y,
                tc.tile_pool(name="p2e", bufs=4) as p2e,
                tc.tile_pool(name="p2acc", bufs=2) as p2acc,
                tc.tile_pool(name="p2r", bufs=2) as p2r,
                tc.tile_pool(name="p2oc", bufs=4) as p2oc,
                tc.tile_pool(name="p2sc", bufs=2, space="PSUM") as p2sc,
                tc.tile_pool(name="p2num", bufs=2, space="PSUM") as p2num,
                tc.tile_pool(name="p2aux", bufs=2, space="PSUM") as p2aux,
            ):
                yts = {}

                def emit_epi(epi):
                    n, h, num, acc = epi
                    den = p2aux.tile([128, SB], F32, tag="aux", name="den")
                    nc.tensor.matmul(
                        den[:], ones_sb[:], acc[:], start=True, stop=True
                    )
                    r = p2r.tile([128, SB], F32, tag="r")
                    nc.vector.reciprocal_approx_fast(out=r[:], in_=den[:])
                    nc.vector.tensor_mul(yts[n][:, h, :], num[:], r[:])

                oproj_count = [0]

                def emit_oproj_group(n, m, oc_i, tail=False):
                    yt = yts[n]
                    mrow = n * SB + m * 128
                    ocb = slice(oc_i * SB, (oc_i + 1) * SB)
                    pso = p2aux.tile([128, SB], F32, tag="aux", name="pso")
                    for kj in range(HPC):
                        nc.tensor.matmul(
                            pso[:],
                            yt[:, kj, m * 128 : (m + 1) * 128],
                            wo[:, kj, ocb],
                            start=(kj == 0),
                            stop=(kj == HPC - 1),
                        )
                    occ = p2oc.tile([128, SB], F16, tag="oc")
                    i = oproj_count[0]
                    oproj_count[0] += 1
                    nc.vector.tensor_scalar_mul(occ[:], pso[:], 1.0)
                    eng = nc.scalar if (tail and i % 2) else nc.sync
                    eng.dma_start(out=out[mrow : mrow + 128, ocb], in_=occ[:])

                pending_epi = None
                oproj_fifo = []  # (n, m, oc_i) groups ready to weave in
                for n in range(NSB):
                    nblk = slice(n * SB, (n + 1) * SB)
                    for h in range(HPC):
                        if h == 0:
                            yts[n] = p2y.tile(
                                [128, HPC, SB], F16, tag="yt", name="yt"
                            )
                        acc_p = p2acc.tile([128, SB], F16, tag="accp")
                        acc_d = p2acc.tile([128, SB], F16, tag="accd")
                        num = p2num.tile([128, SB], F32, tag="num")
                        es = [None] * NP
                        for p in range(NP + PIPE):
                            if p < NP:
                                sc2 = p2sc.tile([128, 2, SB], F32, tag="sc")
                                e2 = p2e.tile([128, 2, SB], F16, tag="e")
                                for half in range(2):
                                    i = 2 * p + half
                                    nc.tensor.matmul(
                                        sc2[:, half, :],
                                        k_sb[:, h, i * 128 : (i + 1) * 128],
                                        q_sb[:, h, nblk],
                                        start=True,
                                        stop=True,
                                    )
                                nc.scalar.activation(
                                    e2[:], sc2[:], AF.Exp, scale=float(SCALE)
                                )
                                es[p] = e2
                                # esum: Pool sums e0-e5, DVE e6-e15; one
                                # merge at block end (late tiles on the
                                # faster engine so den isn't gated on Pool)
                                if p == 0:
                                    nc.gpsimd.tensor_add(
                                        acc_p[:], e2[:, 0, :], e2[:, 1, :]
                                    )
                                elif p <= 2:
                                    for half in range(2):
                                        nc.gpsimd.tensor_add(
                                            acc_p[:], acc_p[:], e2[:, half, :]
                                        )
                                elif p == 3:
                                    nc.vector.tensor_add(
                                        acc_d[:], e2[:, 0, :], e2[:, 1, :]
                                    )
                                else:
                                    for half in range(2):
                                        nc.vector.tensor_add(
                                            acc_d[:], acc_d[:], e2[:, half, :]
                                        )
                            if p == 1 and pending_epi is not None:
                                emit_epi(pending_epi)
                                pending_epi = None
                            if p in (3, 6):
                                for _ in range(2):
                                    if oproj_fifo:
                                        emit_oproj_group(*oproj_fifo.pop(0))
                            if p >= PIPE:
                                pp = p - PIPE
                                for half in range(2):
                                    i = 2 * pp + half
                                    nc.tensor.matmul(
                                        num[:],
                                        v_sb[:, h, i, :],
                                        es[pp][:, half, :],
                                        start=(i == 0),
                                        stop=(i == NKT - 1),
                                    )
                        nc.vector.tensor_add(acc_d[:], acc_d[:], acc_p[:])
                        pending_epi = (n, h, num, acc_d)
                    # after epi(n, h3) is emitted (at the next block's p1),
                    # n's o_proj groups become weavable; enqueue now - the
                    # first pop happens at that block's p3, after the epi
                    oproj_fifo.extend(
                        (n, m, oc_i)
                        for m in range(SB // 128)
                        for oc_i in range(HIDDEN // SB)
                    )

                emit_epi(pending_epi)
                while oproj_fifo:
                    emit_oproj_group(*oproj_fifo.pop(0), tail=True)

    nc.compile()
    return nc


_NC = None


def _get_nc():
    global _NC
    if _NC is None:
        _NC = build()
    return _NC


def _rope_tables():
    inv_freq = 1.0 / (THETA ** (np.arange(0, DH, 2, dtype=np.float32) / DH))
    freqs = np.arange(S, dtype=np.float32)[:, None] * inv_freq[None, :]  # [S, 64]
    cos_h = np.cos(freqs).T.astype(np.float32)  # [64, S]
    sin_h = np.sin(freqs).T.astype(np.float32)
    cos = np.concatenate([cos_h, cos_h], axis=0)  # [128, S]
    sin_s = np.concatenate([-sin_h, sin_h], axis=0)  # [128, S]
    return np.ascontiguousarray(cos), np.ascontiguousarray(sin_s)


def _pm_weight(wT):  # [2048, 512] (k, j) -> [128, 16, 512] partition-major
    return np.ascontiguousarray(
        wT.reshape(KT, 128, JPC).transpose(1, 0, 2)
    ).astype(np.float16)


def _make_in_maps(inputs):
    x = np.asarray(inputs["x"], dtype=np.float32)
    Wq = np.asarray(inputs["Wq"], dtype=np.float32)
    Wk = np.asarray(inputs["Wk"], dtype=np.float32)
    Wv = np.asarray(inputs["Wv"], dtype=np.float32)
    Wo = np.asarray(inputs["Wo"], dtype=np.float32)

    cos, sin_s = _rope_tables()
    cos = cos.astype(np.float16)
    sin_s = sin_s.astype(np.float16)
    ones = np.ones((128, 128), dtype=np.float16)
    pswap = np.zeros((128, 128), dtype=np.float16)
    pswap[(np.arange(128) + 64) % 128, np.arange(128)] = 1.0

    in_maps = []
    for c in range(N_CORES):
        b = c // GROUPS
        g = c % GROUPS
        rows = slice(g * JPC, (g + 1) * JPC)
        xT = x[b].T  # [hidden(k), s]
        # [k, s] -> [s_blk, p, kt, s_in_blk]
        xpm = np.ascontiguousarray(
            xT.reshape(KT, 128, NSB, SB).transpose(2, 1, 0, 3)
        ).astype(np.float16)
        # Wo[:, rows].T -> [512(j), 2048] -> [p, kj, 2048]
        woT = Wo[:, rows].T
        wopm = np.ascontiguousarray(
            woT.reshape(HPC, 128, HIDDEN).transpose(1, 0, 2)
        ).astype(np.float16)
        in_maps.append(
            {
                "xPM": xpm,
                "wqPM": _pm_weight(Wq[rows].T),
                "wkPM": _pm_weight(Wk[rows].T),
                "wvPM": _pm_weight(Wv[rows].T),
                "woPM": wopm,
                "cos": cos,
                "sinS": sin_s,
                "ones": ones,
                "pswap": pswap,
            }
        )
    return in_maps


def kernel(x, Wq, Wk, Wv, Wo):
    nc = _get_nc()
    in_maps = _make_in_maps({"x": x, "Wq": Wq, "Wk": Wk, "Wv": Wv, "Wo": Wo})
    res = run_bass_kernel_spmd(nc, in_maps, list(range(N_CORES)))

    out = np.zeros((B, S, HIDDEN), dtype=np.float32)
    for c in range(N_CORES):
        out[c // GROUPS] += res.results[c]["out"].astype(np.float32)
    return out


# revision 19
# speedup vs baseline: 1.0558x; 1.0181x over previous
"""Trainium2 Bass kernel for nn_BaseTimeAttention (dense transformer block:
QKV projection + RoPE + softmax attention + output projection).

Problem (hardcoded):
  x:  [B=2, S=2048, H=2048] fp32,  Wq/Wk/Wv/Wo: [2048, 2048] fp32
  out = softmax((rope(xWq^T) rope(xWk^T)^T)/sqrt(128)) (xWv^T) Wo^T

Sharding (8 cores): tensor-parallel over heads x data-parallel over batch.
Core c handles batch b=c//4 and head group g=c%4 (4 of 16 heads = 512 of 2048
channels). Each core produces a full [2048, 2048] fp16 partial of the output
projection restricted to its 512 input channels; the host sums 4 partials per
batch in fp32 (o_proj row-parallel reduction on host).

All compute runs in fp16 operands with fp32 PSUM accumulation. Q/K/V live
entirely in SBUF between the projection and attention phases. Inputs stream
on four dedicated DGE rings (x on scalar, wq on gpsimd, wk/wo on sync,
cos/sin/wv on vector) so the startup transient is bandwidth-, not
ordering-limited.

RoPE rotate-half needs a 64-partition swap: for s-block 0 (when the PE is
transfer-starved anyway) it runs as a permutation-matmul on the PE; later
blocks use SBUF->SBUF DMAs on the sync ring. The sign lives in the
host-built sin table.

Attention per (h, n) block (n a 512-wide query block):
  scoresT[s_k, s_q] = K-tile^T @ QT      (16 MMs; pairs share a 2-bank PSUM)
  eT = exp(scoresT / sqrt(128))          (ScalarE, one ACTIVATE per pair)
  num[d, s_q] += V-tile^T @ eT           (PE, accumulate over s_k)
  esum += eT                             (Pool: tiles 0-4, DVE: 5-15 + merge)
  den = ones^T @ esum                    (PE, 1 MM - replaces 16 ones-MMs)
  yt = num * recip(den)                  (VectorE, approx reciprocal)
Block epilogues (den/recip/mul) are software-pipelined into the next block's
score phase, and o_proj work is spread evenly over all blocks (4 PSUM groups
per block, woven in at pipeline steps 3 and 6) so the PE always has runnable
work while the ScalarE exp stream - the co-bottleneck - catches up.
"""

import numpy as np

import concourse.mybir as mybir
import concourse.tile as tile
from concourse import bacc
from concourse.bass_utils import run_bass_kernel_spmd

F32 = mybir.dt.float32
F16 = mybir.dt.float16
AF = mybir.ActivationFunctionType

B = 2
S = 2048
HIDDEN = 2048
HEADS = 16
DH = 128
THETA = 10000.0
N_CORES = 8
GROUPS = 4
HPC = HEADS // GROUPS  # heads per core
JPC = HPC * DH  # projection cols per core
SCALE = 1.0 / np.sqrt(DH)

SB = 512
NSB = S // SB
KT = HIDDEN // 128  # 16 contraction tiles
NKT = S // 128  # 16 s_k tiles
NP = NKT // 2  # 8 score pairs per block
PIPE = 2

CHS = (1, 1, 2, 4, 8)  # phase-1 k-chunk sizes (finest first for fast start)
CHO = (0, 1, 2, 4, 8)


def build():
    nc = bacc.Bacc("TRN2", target_bir_lowering=False, debug=False)

    # partition-major inputs (see _make_in_maps)
    x_d = nc.dram_tensor("xPM", [NSB, 128, KT, SB], F16, kind="ExternalInput")
    wq_d = nc.dram_tensor("wqPM", [128, KT, JPC], F16, kind="ExternalInput")
    wk_d = nc.dram_tensor("wkPM", [128, KT, JPC], F16, kind="ExternalInput")
    wv_d = nc.dram_tensor("wvPM", [128, KT, JPC], F16, kind="ExternalInput")
    wo_d = nc.dram_tensor("woPM", [128, HPC, HIDDEN], F16, kind="ExternalInput")
    cos_d = nc.dram_tensor("cos", [DH, S], F16, kind="ExternalInput")
    sin_d = nc.dram_tensor("sinS", [DH, S], F16, kind="ExternalInput")
    ones_d = nc.dram_tensor("ones", [128, 128], F16, kind="ExternalInput")
    pswap_d = nc.dram_tensor("pswap", [128, 128], F16, kind="ExternalInput")
    out_d = nc.dram_tensor("out", [S, HIDDEN], F16, kind="ExternalOutput")

    out = out_d.ap()

    with tile.TileContext(nc) as tc:
        with tc.tile_pool(name="persist", bufs=1) as persist:
            q_sb = persist.tile([128, HPC, S], F16, tag="q_sb")
            k_sb = persist.tile([128, HPC, S], F16, tag="k_sb")
            v_sb = persist.tile([128, HPC, NKT, DH], F16, tag="v_sb")
            wo = persist.tile([128, HPC, HIDDEN], F16, tag="wo")
            ones_sb = persist.tile([128, 128], F16, tag="ones")
            pswap = persist.tile([128, 128], F16, tag="pswap")
            cos_sb = persist.tile([128, S], F16, tag="cos")
            sin_sb = persist.tile([128, S], F16, tag="sin")
            nc.sync.dma_start(out=ones_sb[:], in_=ones_d.ap())
            nc.sync.dma_start(out=pswap[:], in_=pswap_d.ap())

            # ---------------- Phase 1: projections + RoPE ------------------
            def wslice(chunks, k, cols):
                for off, sz, t in chunks:
                    if off <= k < off + sz:
                        return t[:, k - off, cols]
                raise AssertionError

            pq_cm = tc.tile_pool(name="pq", bufs=1)
            px_cm = tc.tile_pool(name="px", bufs=1)
            pq = pq_cm.__enter__()
            px = px_cm.__enter__()
            with (
                tc.tile_pool(name="p1w", bufs=1) as p1w,
                tc.tile_pool(name="p1s", bufs=4) as p1s,
                tc.tile_pool(name="p1ps", bufs=2, space="PSUM") as p1ps,
                tc.tile_pool(name="p1rot", bufs=2, space="PSUM") as p1rot,
            ):
                # Wave-scheduled loads: x0 and wq (the first-wave inputs)
                # are split across all three DGE rings so the startup ramp is
                # aggregate-bandwidth-bound; wk/wv/wo follow in deadline
                # order. The Tile scheduler interleaves matmul groups
                # chunk-wise, so per-chunk arrival is what matters.
                #   scalar: x0/3rd, cos, wk-h2a, x1, x2, x3
                #   sync:   consts, x0/3rd, wq/3rd, sin, wv-h1 (+swaps, out)
                #   gpsimd: wq/3rd, wk-h1, wv-h2, wo
                def load_split(src_ap, tag, spec, pool, width):
                    lst = []
                    for idx, (eng, off, sz) in enumerate(spec):
                        t = pool.tile(
                            [128, sz, width], F16, tag=f"{tag}{idx}", name=tag
                        )
                        eng.dma_start(out=t[:], in_=src_ap[:, off : off + sz, :])
                        lst.append((off, sz, t))
                    return lst

                wchunks = {}
                xw1 = [
                    (nc.scalar, 0, 1),
                    (nc.scalar, 1, 1),
                    (nc.scalar, 2, 2),
                    (nc.scalar, 4, 2),
                    (nc.sync, 6, 2),
                    (nc.sync, 8, 2),
                    (nc.gpsimd, 10, 3),
                    (nc.gpsimd, 13, 3),
                ]
                wq1 = [
                    (nc.gpsimd, 0, 1),
                    (nc.gpsimd, 1, 1),
                    (nc.gpsimd, 2, 2),
                    (nc.gpsimd, 4, 2),
                    (nc.scalar, 6, 2),
                    (nc.scalar, 8, 2),
                    (nc.sync, 10, 3),
                    (nc.sync, 13, 3),
                ]
                xs0 = load_split(x_d.ap()[0], "xs0", xw1, px, SB)
                wchunks["q"] = load_split(wq_d.ap(), "wq", wq1, pq, JPC)
                nc.scalar.dma_start(out=cos_sb[:], in_=cos_d.ap())
                nc.sync.dma_start(out=sin_sb[:], in_=sin_d.ap())
                wchunks["k"] = load_split(
                    wk_d.ap(),
                    "wk",
                    [
                        (nc.gpsimd, 0, 2),
                        (nc.gpsimd, 2, 2),
                        (nc.gpsimd, 4, 4),
                        (nc.scalar, 8, 4),
                        (nc.sync, 12, 4),
                    ],
                    p1w,
                    JPC,
                )
                wchunks["v"] = load_split(
                    wv_d.ap(),
                    "wv",
                    [
                        (nc.sync, 0, 2),
                        (nc.sync, 2, 2),
                        (nc.sync, 4, 4),
                        (nc.gpsimd, 8, 4),
                        (nc.gpsimd, 12, 4),
                    ],
                    p1w,
                    JPC,
                )
                for kj in range(HPC):
                    nc.gpsimd.dma_start(out=wo[:, kj, :], in_=wo_d.ap()[:, kj, :])

                def load_xs(s):
                    xt = px.tile([128, KT, SB], F16, tag="xsf", bufs=2)
                    nc.scalar.dma_start(out=xt[:], in_=x_d.ap()[s])
                    return [(0, KT, xt)]

                xs_next = xs0

                # s0 uses the PE to produce the rotate-half partner (the PE
                # is transfer-starved early on, so rot-MMs are free); later
                # s-blocks use sync-ring SBUF->SBUF swap DMAs.
                pending_rot = []

                def flush_rot():
                    while pending_rot:
                        qt, dst, j, sblk = pending_rot.pop(0)
                        rot = p1rot.tile([128, SB], F32, tag="rot")
                        nc.tensor.matmul(
                            rot[:], pswap[:], qt[:], start=True, stop=True
                        )
                        t1 = p1s.tile([128, SB], F16, tag="t1")
                        nc.vector.tensor_mul(t1[:], rot[:], sin_sb[:, sblk])
                        nc.vector.tensor_mul(qt[:], qt[:], cos_sb[:, sblk])
                        nc.vector.tensor_add(dst[:, j, sblk], qt[:], t1[:])

                for s in range(NSB):
                    sblk = slice(s * SB, (s + 1) * SB)
                    xsc = xs_next
                    for name, dst in (("q", q_sb), ("k", k_sb), ("v", None)):
                        if name == "q" and s == NSB - 1:
                            xs3_tile = xsc  # deferred: woven into phase 2
                            continue
                        if name == "k" and s + 1 < NSB:
                            xs_next = load_xs(s + 1)
                        for j in range(HPC):
                            jblk = slice(j * 128, (j + 1) * 128)
                            if dst is not None:
                                ps = p1ps.tile([128, SB], F32, tag="ps")
                            else:
                                ps = p1ps.tile(
                                    [128, HPC, DH], F32, tag="ps", name="psv"
                                )
                            for k in range(KT):
                                if dst is not None:  # Q/K: [j, s] transposed
                                    lhsT = wslice(wchunks[name], k, jblk)
                                    rhs = wslice(xsc, k, slice(0, SB))
                                else:  # V: natural [s, j]
                                    lhsT = wslice(xsc, k, jblk)
                                    rhs = wslice(wchunks[name], k, slice(0, JPC))
                                nc.tensor.matmul(
                                    ps[:],
                                    lhsT,
                                    rhs,
                                    start=(k == 0),
                                    stop=(k == KT - 1),
                                )
                            if s == 0:
                                flush_rot()
                            if dst is not None:
                                qt = p1s.tile(
                                    [128, SB], F16, tag="qt", bufs=6
                                )
                                nc.scalar.copy(qt[:], ps[:])
                                if s == 0:
                                    pending_rot.append((qt, dst, j, sblk))
                                else:
                                    tmp = p1s.tile([128, SB], F16, tag="tmp")
                                    nc.sync.dma_start(
                                        out=tmp[0:64, :], in_=qt[64:128, :]
                                    )
                                    nc.sync.dma_start(
                                        out=tmp[64:128, :], in_=qt[0:64, :]
                                    )
                                    nc.vector.tensor_mul(
                                        qt[:], qt[:], cos_sb[:, sblk]
                                    )
                                    nc.vector.tensor_mul(
                                        tmp[:], tmp[:], sin_sb[:, sblk]
                                    )
                                    nc.vector.tensor_add(
                                        dst[:, j, sblk], qt[:], tmp[:]
                                    )
                            else:
                                # V tile covers s rows [s*SB+j*128 ...), all
                                # HPC heads; scatter heads into v_sb
                                nc.scalar.copy(
                                    v_sb[:, :, s * HPC + j, :], ps[:]
                                )
                    if s == 0:
                        flush_rot()

            # ---------------- Phase 2: attention + o_proj ------------------
            with (
                tc.tile_pool(name="p2y", bufs=2) as p2y,
                tc.tile_pool(name="p2s3", bufs=2) as p2s3,
                tc.tile_pool(name="p2e", bufs=4) as p2e,
                tc.tile_pool(name="p2acc", bufs=2) as p2acc,
                tc.tile_pool(name="p2r", bufs=2) as p2r,
                tc.tile_pool(name="p2oc", bufs=4) as p2oc,
                tc.tile_pool(name="p2sc", bufs=2, space="PSUM") as p2sc,
                tc.tile_pool(name="p2num", bufs=2, space="PSUM") as p2num,
                tc.tile_pool(name="p2aux", bufs=2, space="PSUM") as p2aux,
            ):
                yts = {}

                def emit_epi(epi):
                    n, h, num, acc = epi
                    den = p2aux.tile([128, SB], F32, tag="aux", name="den")
                    nc.tensor.matmul(
                        den[:], ones_sb[:], acc[:], start=True, stop=True
                    )
                    r = p2r.tile([128, SB], F32, tag="r")
                    nc.vector.reciprocal_approx_fast(out=r[:], in_=den[:])
                    nc.vector.tensor_mul(yts[n][:, h, :], num[:], r[:])

                oproj_count = [0]

                def emit_qs3_group(j):
                    # deferred q-projection of s-block 3, woven into n0's
                    # attention blocks (which otherwise have no o_proj work
                    # and stall on the exp stream)
                    s3 = NSB - 1
                    sblk = slice(s3 * SB, (s3 + 1) * SB)
                    jblk = slice(j * 128, (j + 1) * 128)
                    ps = p2aux.tile([128, SB], F32, tag="aux", name="psq")
                    for k in range(KT):
                        nc.tensor.matmul(
                            ps[:],
                            wslice(wchunks["q"], k, jblk),
                            wslice(xs3_tile, k, slice(0, SB)),
                            start=(k == 0),
                            stop=(k == KT - 1),
                        )
                    qt = p2s3.tile([128, SB], F16, tag="qt3")
                    tmp = p2s3.tile([128, SB], F16, tag="tmp3")
                    nc.scalar.copy(qt[:], ps[:])
                    nc.sync.dma_start(out=tmp[0:64, :], in_=qt[64:128, :])
                    nc.sync.dma_start(out=tmp[64:128, :], in_=qt[0:64, :])
                    nc.vector.tensor_mul(qt[:], qt[:], cos_sb[:, sblk])
                    nc.vector.tensor_mul(tmp[:], tmp[:], sin_sb[:, sblk])
                    nc.vector.tensor_add(q_sb[:, j, sblk], qt[:], tmp[:])

                def emit_oproj_group(n, m, oc_i, tail=False):
                    yt = yts[n]
                    mrow = n * SB + m * 128
                    ocb = slice(oc_i * SB, (oc_i + 1) * SB)
                    pso = p2aux.tile([128, SB], F32, tag="aux", name="pso")
                    for kj in range(HPC):
                        nc.tensor.matmul(
                            pso[:],
                            yt[:, kj, m * 128 : (m + 1) * 128],
                            wo[:, kj, ocb],
                            start=(kj == 0),
                            stop=(kj == HPC - 1),
                        )
                    occ = p2oc.tile([128, SB], F16, tag="oc")
                    i = oproj_count[0]
                    oproj_count[0] += 1
                    if tail and i % 2 == 0:
                        nc.scalar.copy(occ[:], pso[:])
                    else:
                        nc.vector.tensor_scalar_mul(occ[:], pso[:], 1.0)
                    eng = nc.scalar if (tail and i % 2) else nc.sync
                    eng.dma_start(out=out[mrow : mrow + 128, ocb], in_=occ[:])

                pending_epi = None
                oproj_fifo = []  # (n, m, oc_i) groups ready to weave in
                for n in range(NSB):
                    nblk = slice(n * SB, (n + 1) * SB)
                    for h in range(HPC):
                        if h == 0:
                            yts[n] = p2y.tile(
                                [128, HPC, SB], F16, tag="yt", name="yt"
                            )
                        acc_p = p2acc.tile([128, SB], F16, tag="accp")
                        acc_d = p2acc.tile([128, SB], F16, tag="accd")
                        num = p2num.tile([128, SB], F32, tag="num")
                        es = [None] * NP
                        for p in range(NP + PIPE):
                            if p < NP:
                                sc2 = p2sc.tile([128, 2, SB], F32, tag="sc")
                                e2 = p2e.tile([128, 2, SB], F16, tag="e")
                                for half in range(2):
                                    i = 2 * p + half
                                    nc.tensor.matmul(
                                        sc2[:, half, :],
                                        k_sb[:, h, i * 128 : (i + 1) * 128],
                                        q_sb[:, h, nblk],
                                        start=True,
                                        stop=True,
                                    )
                                nc.scalar.activation(
                                    e2[:], sc2[:], AF.Exp, scale=float(SCALE)
                                )
                                es[p] = e2
                                # esum: Pool sums e0-e5, DVE e6-e15; one
                                # merge at block end (late tiles on the
                                # faster engine so den isn't gated on Pool)
                                if p == 0:
                                    nc.gpsimd.tensor_add(
                                        acc_p[:], e2[:, 0, :], e2[:, 1, :]
                                    )
                                elif p <= 2:
                                    for half in range(2):
                                        nc.gpsimd.tensor_add(
                                            acc_p[:], acc_p[:], e2[:, half, :]
                                        )
                                elif p == 3:
                                    nc.vector.tensor_add(
                                        acc_d[:], e2[:, 0, :], e2[:, 1, :]
                                    )
                                else:
                                    for half in range(2):
                                        nc.vector.tensor_add(
                                            acc_d[:], acc_d[:], e2[:, half, :]
                                        )
                            if p == 1 and pending_epi is not None:
                                emit_epi(pending_epi)
                                pending_epi = None
                            if p == 3 and n == 0:
                                emit_qs3_group(h)
                            if p in (3, 6):
                                for _ in range(2):
                                    if oproj_fifo:
                                        emit_oproj_group(*oproj_fifo.pop(0))
                            if p >= PIPE:
                                pp = p - PIPE
                                for half in range(2):
                                    i = 2 * pp + half
                                    nc.tensor.matmul(
                                        num[:],
                                        v_sb[:, h, i, :],
                                        es[pp][:, half, :],
                                        start=(i == 0),
                                        stop=(i == NKT - 1),
                                    )
                        nc.vector.tensor_add(acc_d[:], acc_d[:], acc_p[:])
                        pending_epi = (n, h, num, acc_d)
                    # after epi(n, h3) is emitted (at the next block's p1),
                    # n's o_proj groups become weavable; enqueue now - the
                    # first pop happens at that block's p3, after the epi
                    oproj_fifo.extend(
                        (n, m, oc_i)
                        for m in range(SB // 128)
                        for oc_i in range(HIDDEN // SB)
                    )

                emit_epi(pending_epi)
                while oproj_fifo:
                    emit_oproj_group(*oproj_fifo.pop(0), tail=True)

            px_cm.__exit__(None, None, None)
            pq_cm.__exit__(None, None, None)

    nc.compile()
    return nc


_NC = None


def _get_nc():
    global _NC
    if _NC is None:
        _NC = build()
    return _NC


def _rope_tables():
    inv_freq = 1.0 / (THETA ** (np.arange(0, DH, 2, dtype=np.float32) / DH))
    freqs = np.arange(S, dtype=np.float32)[:, None] * inv_freq[None, :]  # [S, 64]
    cos_h = np.cos(freqs).T.astype(np.float32)  # [64, S]
    sin_h = np.sin(freqs).T.astype(np.float32)
    cos = np.concatenate([cos_h, cos_h], axis=0)  # [128, S]
    sin_s = np.concatenate([-sin_h, sin_h], axis=0)  # [128, S]
    return np.ascontiguousarray(cos), np.ascontiguousarray(sin_s)


def _pm_weight(wT):  # [2048, 512] (k, j) -> [128, 16, 512] partition-major
    return np.ascontiguousarray(
        wT.reshape(KT, 128, JPC).transpose(1, 0, 2)
    ).astype(np.float16)


def _make_in_maps(inputs):
    x = np.asarray(inputs["x"], dtype=np.float32)
    Wq = np.asarray(inputs["Wq"], dtype=np.float32)
    Wk = np.asarray(inputs["Wk"], dtype=np.float32)
    Wv = np.asarray(inputs["Wv"], dtype=np.float32)
    Wo = np.asarray(inputs["Wo"], dtype=np.float32)

    cos, sin_s = _rope_tables()
    cos = cos.astype(np.float16)
    sin_s = sin_s.astype(np.float16)
    ones = np.ones((128, 128), dtype=np.float16)
    pswap = np.zeros((128, 128), dtype=np.float16)
    pswap[(np.arange(128) + 64) % 128, np.arange(128)] = 1.0

    in_maps = []
    for c in range(N_CORES):
        b = c // GROUPS
        g = c % GROUPS
        rows = slice(g * JPC, (g + 1) * JPC)
        xT = x[b].T  # [hidden(k), s]
        # [k, s] -> [s_blk, p, kt, s_in_blk]
        xpm = np.ascontiguousarray(
            xT.reshape(KT, 128, NSB, SB).transpose(2, 1, 0, 3)
        ).astype(np.float16)
        # Wo[:, rows].T -> [512(j), 2048] -> [p, kj, 2048]
        woT = Wo[:, rows].T
        wopm = np.ascontiguousarray(
            woT.reshape(HPC, 128, HIDDEN).transpose(1, 0, 2)
        ).astype(np.float16)
        in_maps.append(
            {
                "xPM": xpm,
                "wqPM": _pm_weight(Wq[rows].T),
                "wkPM": _pm_weight(Wk[rows].T),
                "wvPM": _pm_weight(Wv[rows].T),
                "woPM": wopm,
                "cos": cos,
                "sinS": sin_s,
                "ones": ones,
                "pswap": pswap,
            }
        )
    return in_maps


def kernel(x, Wq, Wk, Wv, Wo):
    nc = _get_nc()
    in_maps = _make_in_maps({"x": x, "Wq": Wq, "Wk": Wk, "Wv": Wv, "Wo": Wo})
    res = run_bass_kernel_spmd(nc, in_maps, list(range(N_CORES)))

    out = np.zeros((B, S, HIDDEN), dtype=np.float32)
    for c in range(N_CORES):
        out[c // GROUPS] += res.results[c]["out"].astype(np.float32)
    return out
